# revision 1
# baseline (speedup 1.0000x reference)
"""Trainium2 Bass kernel for nn_DinoGazeSpade (segment_reduce + SPADE stack).

Layout: 8 cores; image k = core//2; each core computes rows [16h, 16h+16) of
the 32x32 grid (h = core%2). Cross-core: 3 pairwise AllReduces of LayerNorm
partial stats. Heavy convs in fp16 matmuls, fp32 accumulate.

Key algebra:
  - painted map (448x448) never materialized: bilinear 448->32 samples exactly
    4 seg pixels per output at weight 1/4, so sm is the per-segment means
    avg[64,384] gathered through corner-count matrices; scatter_mean and gather
    are both matmuls against one-hot masks built via is_equal(iota, ids).
  - SPADE0's wb conv (128->1536) folded through conv0_w (1x1, 1536->8) on the
    host into a 128->8 conv. Same for SPADE1/2 wb convs.
  - LayerNorm linearized through the 1x1 convs: out = softplus(r*A + (-mu*r)*B
    + C + b) with A = W@(x .* gp1), B = W@gp1, C = fold(h) all independent of
    the stats, so every heavy matmul is emitted before any collective-dependent
    PE op (the PE queue is in-order; this hides the AllReduce latency).
  - LN stats via bn_stats/bn_aggr; rsqrt as exp(-0.5*ln(var+eps)) so every
    activation fits one ACT table set.
"""
import os
import numpy as np
from contextlib import ExitStack

import concourse.bass as bass
import concourse.mybir as mybir
import concourse.tile as tile
from concourse import bacc
from concourse.bass_utils import run_bass_kernel_spmd
from concourse.masks import make_identity

f32 = mybir.dt.float32
f16 = mybir.dt.float16
AF = mybir.ActivationFunctionType
ALU = mybir.AluOpType
AX = mybir.AxisListType

NSEG = 64
B, Cd, Hp, Wp, H, W, Cm, HID = 4, 384, 32, 32, 448, 448, 1536, 128
NPOS = Hp * Wp          # 1024
HROWS = 16              # rows per core
SMR = HROWS + 4         # sm rows incl 2-halo each side = 20
HR = HROWS + 2          # h rows incl 1-halo each side = 18
SMW = 34                # padded width

LAST_RESULTS = None  # set by kernel() for test harness introspection

_BUILT = None

TAPS = [(t // 3, t % 3) for t in range(9)]


def _softplus(nc, pool, z_in, bias_ap, out_tile, p, n, tag):
    """out = softplus(z_in + bias) = relu(z) + ln(1+exp(-|z|)) exactly."""
    t_abs = pool.tile([p, n], f32, tag="sp_abs", name=f"abs{tag}")
    nc.scalar.activation(out=t_abs, in_=z_in, func=AF.Abs, bias=bias_ap)
    t_exp = pool.tile([p, n], f32, tag="sp_exp", name=f"exp{tag}")
    nc.scalar.activation(out=t_exp, in_=t_abs, func=AF.Exp, scale=-1.0)
    t_ln = pool.tile([p, n], f32, tag="sp_ln", name=f"ln{tag}")
    nc.scalar.activation(out=t_ln, in_=t_exp, func=AF.Ln, bias=1.0)
    t_relu = pool.tile([p, n], f32, tag="sp_relu", name=f"relu{tag}")
    nc.scalar.activation(out=t_relu, in_=z_in, func=AF.Relu, bias=bias_ap)
    nc.vector.tensor_tensor(out=out_tile, in0=t_ln, in1=t_relu, op=ALU.add)


def _ln_finish(nc, pool, pst, work, n_inst, st_l, st_g, gid):
    """pst [2,1] = partial (sum of per-partition means, sum of E[x^2]).
    AllReduce over the pair -> r = 1/sqrt(var+eps), -mu*r in work[:, 5:7]."""
    st_sb = pool.tile([2, 1], f32, tag=f"st_sb{gid}", name=f"st_sb{gid}")
    nc.scalar.copy(st_sb, pst)
    nc.sync.dma_start(out=st_l[:], in_=st_sb[0:2, 0:1])
    nc.gpsimd.collective_compute(
        "AllReduce", ALU.add,
        replica_groups=[[0, 1], [2, 3], [4, 5], [6, 7]],
        ins=[st_l[:]], outs=[st_g[:]],
    )
    stg = pool.tile([1, 2], f32, tag=f"stg{gid}", name=f"stg{gid}")
    nc.sync.dma_start(out=stg, in_=st_g[None, :])
    nc.vector.tensor_scalar_mul(work[:, 0:2], stg[:, 0:2], 1.0 / n_inst)   # mu, E[x^2]
    nc.vector.tensor_tensor(out=work[:, 2:3], in0=work[:, 0:1], in1=work[:, 0:1], op=ALU.mult)
    nc.vector.tensor_tensor(out=work[:, 3:4], in0=work[:, 1:2], in1=work[:, 2:3], op=ALU.subtract)
    nc.scalar.activation(out=work[:, 4:5], in_=work[:, 3:4], func=AF.Ln, bias=1e-12)
    nc.scalar.activation(out=work[:, 5:6], in_=work[:, 4:5], func=AF.Exp, scale=-0.5)
    nc.vector.tensor_tensor(out=work[:, 7:8], in0=work[:, 0:1], in1=work[:, 5:6], op=ALU.mult)
    nc.vector.tensor_scalar_mul(work[:, 6:7], work[:, 7:8], -1.0)          # -mu*r


def _bn_partial(nc, pool, src, p, nchunks, tag):
    """bn_stats over src[p, nchunks, 512] -> mv[p,2] = (mean, E[x^2])."""
    bno = pool.tile([p, nchunks, 6], f32, tag=f"bno{tag}", name=f"bno{tag}")
    for kc in range(nchunks):
        nc.vector.bn_stats(out=bno[:, kc, :], in_=src[:, kc, :])
    mv = pool.tile([p, 2], f32, tag=f"mv{tag}", name=f"mv{tag}")
    nc.vector.bn_aggr(out=mv, in_=bno)
    m2 = pool.tile([p, 1], f32, tag=f"m2{tag}", name=f"m2{tag}")
    nc.vector.tensor_tensor(out=m2, in0=mv[:, 0:1], in1=mv[:, 0:1], op=ALU.mult)
    nc.vector.tensor_tensor(out=mv[:, 1:2], in0=mv[:, 1:2], in1=m2, op=ALU.add)
    return mv


def _build_nc():
    nc = bacc.Bacc("TRN2", num_devices=8)

    for val in (1e-12,):
        t = nc.alloc_sbuf_tensor(f"const-float32-{val}", [128, 1], f32)
        nc.gpsimd.memset(t.ap(), val)
        nc.const_aps.aps[(f32, val)] = t.ap()
    nc.all_engine_barrier()

    # ---------------- DRAM I/O ----------------
    d_x = nc.dram_tensor("x", [128, 12, 512], f16, kind="ExternalInput")
    d_ft = nc.dram_tensor("ft", [128, 8, 384], f16, kind="ExternalInput")
    d_ids = nc.dram_tensor("ids", [128, 8], f32, kind="ExternalInput")
    d_cid = nc.dram_tensor("cid", [128, 5, 4], f32, kind="ExternalInput")
    d_hmask = nc.dram_tensor("hmask", [HR], f16, kind="ExternalInput")
    d_ws = nc.dram_tensor("ws", [128, 3, 3, 9, 128], f16, kind="ExternalInput")
    d_wg = nc.dram_tensor("wg", [128, 12, 9, 128], f16, kind="ExternalInput")
    # wsm9 last-axis concat: wf0(8), wg1(8), wf1(16), wg2(16), wf2(1)
    d_wsm9 = nc.dram_tensor("wsm9", [128, 9, 49], f16, kind="ExternalInput")
    d_w0t = nc.dram_tensor("w0t", [128, 12, 8], f16, kind="ExternalInput")
    d_wsm = nc.dram_tensor("wsm", [144], f16, kind="ExternalInput")  # w1t|w2t
    d_bs = nc.dram_tensor("bs", [128, 3], f32, kind="ExternalInput")
    d_gb0 = nc.dram_tensor("gb0", [128, 12], f32, kind="ExternalInput")
    # biasv: gb1(8), gb2(16), b0f(8), b1f(16), b2f(1)
    d_biasv = nc.dram_tensor("biasv", [49], f32, kind="ExternalInput")
    d_out = nc.dram_tensor("out_half", [512], f32, kind="ExternalOutput")

    st_l = [nc.dram_tensor(f"st{i}_l", [2], f32) for i in range(3)]
    st_g = [nc.dram_tensor(f"st{i}_g", [2], f32) for i in range(3)]

    with ExitStack() as ctx:
        tc = ctx.enter_context(tile.TileContext(nc, num_cores=8))
        cpool = ctx.enter_context(tc.tile_pool(name="consts", bufs=1))
        dpool = ctx.enter_context(tc.tile_pool(name="data", bufs=1))
        spool = ctx.enter_context(tc.tile_pool(name="small", bufs=1))
        ps = ctx.enter_context(tc.tile_pool(name="ps", bufs=1, space="PSUM"))

        def MAIN(shape, name):
            return ps.tile(shape, f32, tag="ps_main", bufs=3, name=name)

        def ABC(name):
            return ps.tile([16, 512], f32, tag="ps_abc", bufs=3, name=name)

        def MISC(shape, dt, name):
            return ps.tile(shape, dt, tag="ps_misc", bufs=1, name=name)

        # ---- gpsimd constants first (iota gates the OH build) ----
        iot = cpool.tile([128, 64], f32)
        nc.gpsimd.iota(iot, pattern=[[1, 64]], base=0, channel_multiplier=0,
                       allow_small_or_imprecise_dtypes=True)
        ident = cpool.tile([128, 128], f16)
        make_identity(nc, ident)
        ones_col = cpool.tile([128, 1], f32)
        nc.gpsimd.memset(ones_col, 1.0)
        ones_row = cpool.tile([1, 128], f32)
        nc.gpsimd.memset(ones_row, 1.0)

        # --------- DMAs, ordered so early-needed data lands first ---------
        idst = cpool.tile([128, 8], f32)
        nc.sync.dma_start(out=idst, in_=d_ids[:, :])
        cidt = cpool.tile([128, 5, 4], f32)
        nc.sync.dma_start(out=cidt, in_=d_cid[:, :, :])
        feats = dpool.tile([128, 8, 385], f16)
        nc.sync.dma_start(out=feats[:, 0:4, 0:384], in_=d_ft[:, 0:4, :])
        nc.sync.dma_start(out=feats[:, 4:8, 0:384], in_=d_ft[:, 4:8, :])
        bs_t = cpool.tile([128, 3], f32)
        nc.sync.dma_start(out=bs_t, in_=d_bs[:, :])
        ws_t = cpool.tile([128, 3, 3, 9, 128], f16)
        nc.sync.dma_start(out=ws_t[:, 0:1], in_=d_ws[:, 0:1])     # s0_ws first
        gb0_t = cpool.tile([128, 12], f32)
        nc.sync.dma_start(out=gb0_t, in_=d_gb0[:, :])
        xt = dpool.tile([128, 12, 512], f16)
        nc.sync.dma_start(out=xt, in_=d_x[:, :, :])
        wg_t = cpool.tile([128, 12, 9, 128], f16)
        for g in range(3):
            nc.sync.dma_start(out=wg_t[:, g * 4:(g + 1) * 4], in_=d_wg[:, g * 4:(g + 1) * 4])
        nc.sync.dma_start(out=ws_t[:, 1:3], in_=d_ws[:, 1:3])     # s1/s2_ws
        w0t_t = cpool.tile([128, 12, 8], f16)
        nc.sync.dma_start(out=w0t_t, in_=d_w0t[:, :, :])
        wsm9_t = cpool.tile([128, 9, 49], f16)
        nc.sync.dma_start(out=wsm9_t, in_=d_wsm9[:, :, :])
        wf0_t = wsm9_t[:, :, 0:8]
        wg1_t = wsm9_t[:, :, 8:16]
        wf1_t = wsm9_t[:, :, 16:32]
        wg2_t = wsm9_t[:, :, 32:48]
        wf2_t = wsm9_t[:, :, 48:49]
        w1t_t = cpool.tile([8, 16], f16)
        nc.sync.dma_start(out=w1t_t, in_=d_wsm[0:128].rearrange("(a b) -> a b", b=16))
        w2t_t = cpool.tile([16, 1], f16)
        nc.sync.dma_start(out=w2t_t, in_=d_wsm[128:144][:, None])
        gb1_t = cpool.tile([8, 1], f32)
        nc.sync.dma_start(out=gb1_t, in_=d_biasv[0:8][:, None])
        gb2b = cpool.tile([16, 1], f32)
        nc.sync.dma_start(out=gb2b, in_=d_biasv[8:24][:, None])
        b0fb = cpool.tile([8, 1], f32)
        nc.sync.dma_start(out=b0fb, in_=d_biasv[24:32][:, None])
        b1fb = cpool.tile([16, 1], f32)
        nc.sync.dma_start(out=b1fb, in_=d_biasv[32:48][:, None])
        b2fb = cpool.tile([1, 1], f32)
        nc.sync.dma_start(out=b2fb, in_=d_biasv[48:49][:, None])
        hmask_bc = cpool.tile([128, HR], f16)
        nc.gpsimd.dma_start(out=hmask_bc, in_=d_hmask[None, :].to_broadcast([128, HR]))

        nc.gpsimd.memset(feats[:, :, 384:385], 1.0)
        # ---------------- segment means avg' [64, 384] ----------------
        oh_t = dpool.tile([128, 8, 64], f16)
        for qc in range(8):
            nc.vector.tensor_scalar(out=oh_t[:, qc, :], in0=iot,
                                    scalar1=idst[:, qc:qc + 1], scalar2=None,
                                    op0=ALU.is_equal)
        psums = ps.tile([64, 385], f32, tag="ps_sums", bufs=1)
        for qc in range(8):
            nc.tensor.matmul(psums, oh_t[:, qc, :], feats[:, qc, :],
                             start=(qc == 0), stop=(qc == 7))
        cnt4 = spool.tile([64, 1], f32, tag="cnt4")
        nc.vector.tensor_scalar(out=cnt4, in0=psums[:, 384:385], scalar1=1.0,
                                scalar2=4.0, op0=ALU.max, op1=ALU.mult)
        recip4 = spool.tile([64, 1], f32, tag="recip4")
        nc.vector.reciprocal(out=recip4, in_=cnt4)
        avg_t = dpool.tile([64, 384], f16)
        nc.vector.tensor_scalar_mul(avg_t, psums[:, 0:384], recip4[:, 0:1])

        # ---------------- G masks -> Gr [64, 640] ----------------
        gacc = dpool.tile([128, 5, 64], f16)
        gtmp = dpool.tile([128, 64], f16)
        for jc in range(5):
            nc.vector.tensor_scalar(out=gacc[:, jc, :], in0=iot,
                                    scalar1=cidt[:, jc, 0:1], scalar2=None,
                                    op0=ALU.is_equal)
            for corner in range(1, 4):
                nc.vector.tensor_scalar(out=gtmp, in0=iot,
                                        scalar1=cidt[:, jc, corner:corner + 1],
                                        scalar2=None, op0=ALU.is_equal)
                nc.vector.tensor_tensor(out=gacc[:, jc, :], in0=gacc[:, jc, :],
                                        in1=gtmp, op=ALU.add)
        gr_t = dpool.tile([64, 640], f16)
        for jc in range(5):
            ptr = MISC([64, 128], f16, f"ptr{jc}")
            nc.tensor.transpose(ptr, gacc[:, jc, :], ident)
            nc.scalar.copy(gr_t[:, jc * 128:(jc + 1) * 128], ptr)

        # ---------------- sm ----------------
        sm_pad = dpool.tile([128, 3, SMR, SMW], f16)
        nc.gpsimd.memset(sm_pad, 0.0)
        for mc in range(3):
            for nch in range(2):
                psm = MAIN([128, 320], f"psm{mc}{nch}")
                nc.tensor.matmul(psm, avg_t[:, mc * 128:(mc + 1) * 128],
                                 gr_t[:, nch * 320:(nch + 1) * 320],
                                 start=True, stop=True)
                nc.scalar.copy(sm_pad[:, mc, nch * 10:(nch + 1) * 10, 1:33],
                               psm.rearrange("p (r c) -> p r c", c=32))

        # ---------------- h conv helper ----------------
        def h_conv(cv):
            hp = dpool.tile([128, HR, SMW], f16, tag=f"hpad{cv}", name=f"hpad{cv}")
            nc.gpsimd.memset(hp, 0.0)
            for nch in range(2):
                psh = MAIN([128, 9 * 32], f"psh{cv}{nch}")
                for kc in range(3):
                    for t, (dy, dx) in enumerate(TAPS):
                        r0 = nch * 9 + dy
                        nc.tensor.matmul(
                            psh, ws_t[:, cv, kc, t, :],
                            sm_pad[:, kc, r0:r0 + 9, dx:dx + 32],
                            start=(kc == 0 and t == 0), stop=(kc == 2 and t == 8))
                nc.scalar.activation(
                    out=hp[:, nch * 9:(nch + 1) * 9, 1:33],
                    in_=psh.rearrange("p (r c) -> p r c", c=32),
                    func=AF.Relu, bias=bs_t[:, cv:cv + 1])
            nc.vector.tensor_tensor(
                out=hp, in0=hp,
                in1=hmask_bc[:, :, None].to_broadcast([128, HR, SMW]),
                op=ALU.mult)
            return hp

        h0p = h_conv(0)

        # ---------------- LN0 partial stats + collective (off PE path) ------
        mv0 = _bn_partial(nc, dpool, xt, 128, 12, "0")
        pst0 = MISC([2, 1], f32, "pst0")
        nc.tensor.matmul(pst0, mv0, ones_col, start=True, stop=True)
        work0 = spool.tile([1, 8], f32, tag="work0")
        _ln_finish(nc, spool, pst0, work0, 256.0, st_l[0], st_g[0], 0)

        # ---------------- conv_g + xg/gp1; A0/B0/C0 ----------------
        gp1 = dpool.tile([128, 12, 512], f16)
        xg = dpool.tile([128, 12, 512], f16)
        psA0 = ABC("psA0")
        psB0 = ABC("psB0")
        for kc in range(12):
            psg = MAIN([128, 512], f"psg{kc}")
            for t, (dy, dx) in enumerate(TAPS):
                nc.tensor.matmul(psg, wg_t[:, kc, t, :],
                                 h0p[:, dy:dy + 16, dx:dx + 32],
                                 start=(t == 0), stop=(t == 8))
            nc.scalar.activation(out=gp1[:, kc, :], in_=psg, func=AF.Identity,
                                 bias=gb0_t[:, kc:kc + 1])
            nc.vector.tensor_tensor(out=xg[:, kc, :], in0=xt[:, kc, :],
                                    in1=gp1[:, kc, :], op=ALU.mult)
        for kc in range(12):
            nc.tensor.matmul(psA0[0:8, :], w0t_t[:, kc, :], xg[:, kc, :],
                             start=(kc == 0), stop=(kc == 11))
        for kc in range(12):
            nc.tensor.matmul(psB0[0:8, :], w0t_t[:, kc, :], gp1[:, kc, :],
                             start=(kc == 0), stop=(kc == 11))
        psC0 = ABC("psC0")
        for t, (dy, dx) in enumerate(TAPS):
            nc.tensor.matmul(psC0[0:8, :], wf0_t[:, t, :],
                             h0p[:, dy:dy + 16, dx:dx + 32],
                             start=(t == 0), stop=(t == 8))

        # broadcast r0 / -mu0*r0 to 8 partitions (PE op, after CC0)
        pbc0 = MISC([8, 2], f32, "pbc0")
        nc.tensor.matmul(pbc0, ones_row[:, 0:8], work0[:, 5:7], start=True, stop=True)
        rbc0 = spool.tile([8, 2], f32, tag="rbc0")
        nc.scalar.copy(rbc0, pbc0)
        # z0 = r0*A0 + (-mu0*r0)*B0 + C0 ; out0 = softplus(z0 + b0f)
        z0 = dpool.tile([8, 512], f32, name="z0")
        zt0 = dpool.tile([8, 512], f32, name="zt0")
        nc.vector.tensor_scalar_mul(z0, psA0[0:8, :], rbc0[:, 0:1])
        nc.vector.tensor_scalar_mul(zt0, psB0[0:8, :], rbc0[:, 1:2])
        nc.vector.tensor_tensor(out=z0, in0=z0, in1=zt0, op=ALU.add)
        nc.vector.tensor_tensor(out=z0, in0=z0, in1=psC0[0:8, :], op=ALU.add)
        out0 = dpool.tile([8, 512], f32)
        _softplus(nc, dpool, z0, b0fb[:, 0:1], out0, 8, 512, "0")

        # ---------------- LN1 partial + collective ----------------
        mv1 = _bn_partial(nc, spool, out0[:, None, :], 8, 1, "1")
        pst1 = MISC([2, 1], f32, "pst1")
        nc.tensor.matmul(pst1, mv1, ones_col[0:8, :], start=True, stop=True)
        work1 = spool.tile([1, 8], f32, tag="work1")
        _ln_finish(nc, spool, pst1, work1, 16.0, st_l[1], st_g[1], 1)

        # PE work that fills the CC1 window
        h1p = h_conv(1)
        h2p = h_conv(2)
        psg1 = ABC("psg1")
        for t, (dy, dx) in enumerate(TAPS):
            nc.tensor.matmul(psg1[0:8, :], wg1_t[:, t, :],
                             h1p[:, dy:dy + 16, dx:dx + 32],
                             start=(t == 0), stop=(t == 8))
        gp11 = spool.tile([8, 512], f16, tag="gp11")
        nc.scalar.activation(out=gp11, in_=psg1[0:8, :], func=AF.Identity,
                             bias=gb1_t[:, 0:1])
        og1 = spool.tile([8, 512], f16, tag="og1")
        nc.vector.tensor_tensor(out=og1, in0=out0, in1=gp11, op=ALU.mult)
        psA1 = ABC("psA1")
        nc.tensor.matmul(psA1, w1t_t, og1, start=True, stop=True)
        psB1 = ABC("psB1")
        nc.tensor.matmul(psB1, w1t_t, gp11, start=True, stop=True)
        psC1 = ABC("psC1")
        for t, (dy, dx) in enumerate(TAPS):
            nc.tensor.matmul(psC1, wf1_t[:, t, :],
                             h1p[:, dy:dy + 16, dx:dx + 32],
                             start=(t == 0), stop=(t == 8))

        pbc1 = MISC([16, 2], f32, "pbc1")
        nc.tensor.matmul(pbc1, ones_row[:, 0:16], work1[:, 5:7], start=True, stop=True)
        rbc1 = spool.tile([16, 2], f32, tag="rbc1")
        nc.scalar.copy(rbc1, pbc1)
        z1 = dpool.tile([16, 512], f32, name="z1")
        zt1 = dpool.tile([16, 512], f32, name="zt1")
        nc.vector.tensor_scalar_mul(z1, psA1, rbc1[:, 0:1])
        nc.vector.tensor_scalar_mul(zt1, psB1, rbc1[:, 1:2])
        nc.vector.tensor_tensor(out=z1, in0=z1, in1=zt1, op=ALU.add)
        nc.vector.tensor_tensor(out=z1, in0=z1, in1=psC1, op=ALU.add)
        out1 = dpool.tile([16, 512], f32)
        _softplus(nc, dpool, z1, b1fb[:, 0:1], out1, 16, 512, "1")

        # ---------------- LN2 partial + collective ----------------
        mv2 = _bn_partial(nc, spool, out1[:, None, :], 16, 1, "2")
        pst2 = MISC([2, 1], f32, "pst2")
        nc.tensor.matmul(pst2, mv2, ones_col[0:16, :], start=True, stop=True)
        work2 = spool.tile([1, 8], f32, tag="work2")
        _ln_finish(nc, spool, pst2, work2, 32.0, st_l[2], st_g[2], 2)

        psg2 = ABC("psg2")
        for t, (dy, dx) in enumerate(TAPS):
            nc.tensor.matmul(psg2, wg2_t[:, t, :],
                             h2p[:, dy:dy + 16, dx:dx + 32],
                             start=(t == 0), stop=(t == 8))
        gp12 = spool.tile([16, 512], f16, tag="gp12")
        nc.scalar.activation(out=gp12, in_=psg2, func=AF.Identity,
                             bias=gb2b[:, 0:1])
        og2 = spool.tile([16, 512], f16, tag="og2")
        nc.vector.tensor_tensor(out=og2, in0=out1, in1=gp12, op=ALU.mult)
        psA2 = ABC("psA2")
        nc.tensor.matmul(psA2[0:1, :], w2t_t, og2, start=True, stop=True)
        psB2 = ABC("psB2")
        nc.tensor.matmul(psB2[0:1, :], w2t_t, gp12, start=True, stop=True)
        psC2 = ABC("psC2")
        for t, (dy, dx) in enumerate(TAPS):
            nc.tensor.matmul(psC2[0:1, :], wf2_t[:, t, :],
                             h2p[:, dy:dy + 16, dx:dx + 32],
                             start=(t == 0), stop=(t == 8))

        # final combine: scalars live on partition 0 -> no broadcast needed
        z2 = dpool.tile([1, 512], f32, name="z2")
        zt2 = dpool.tile([1, 512], f32, name="zt2")
        nc.vector.tensor_scalar_mul(z2, psA2[0:1, :], work2[:, 5:6])
        nc.vector.tensor_scalar_mul(zt2, psB2[0:1, :], work2[:, 6:7])
        nc.vector.tensor_tensor(out=z2, in0=z2, in1=zt2, op=ALU.add)
        nc.vector.tensor_tensor(out=z2, in0=z2, in1=psC2[0:1, :], op=ALU.add)
        final = dpool.tile([1, 512], f32)
        _softplus(nc, dpool, z2, b2fb[:, 0:1], final, 1, 512, "2")
        nc.sync.dma_start(out=d_out[:], in_=final[0:1, :])

    nc.compile()
    return nc


def _host_prep(inputs):
    """Build per-core in_maps (host work: slicing, layout, small weight folds)."""
    x_main = np.asarray(inputs["x_main"], np.float32)
    f_sem = np.asarray(inputs["f_sem"], np.float32)
    seg = np.asarray(inputs["seg_mask"])

    def lhsT9(w):  # [O, I, 3, 3] -> [I, 9, O]
        return np.ascontiguousarray(w.transpose(1, 2, 3, 0).reshape(w.shape[1], 9, w.shape[0]))

    ws_stack = np.stack([inputs["s0_ws"], inputs["s1_ws"], inputs["s2_ws"]])  # [3,128,384,3,3]
    ws_r = ws_stack.reshape(3, 128, 3, 128, 3, 3)          # cv, o, kc, i, ky, kx
    WS = np.ascontiguousarray(ws_r.transpose(3, 0, 2, 4, 5, 1)
                              .reshape(128, 3, 3, 9, 128)).astype(np.float16)
    wg0 = np.asarray(inputs["s0_wg"], np.float32)          # [1536, 128, 3, 3]
    WG = np.ascontiguousarray(
        wg0.reshape(12, 128, 128, 3, 3).transpose(2, 0, 3, 4, 1)
        .reshape(128, 12, 9, 128)).astype(np.float16)
    wf0 = np.einsum("oc,cikl->oikl", np.asarray(inputs["conv0_w"], np.float64),
                    np.asarray(inputs["s0_wb"], np.float64))
    wf1 = np.einsum("oc,cikl->oikl", np.asarray(inputs["conv1_w"], np.float64),
                    np.asarray(inputs["s1_wb"], np.float64))
    wf2 = np.einsum("oc,cikl->oikl", np.asarray(inputs["conv2_w"], np.float64),
                    np.asarray(inputs["s2_wb"], np.float64))
    WSM9 = np.concatenate([
        lhsT9(wf0), lhsT9(np.asarray(inputs["s1_wg"], np.float64)),
        lhsT9(wf1), lhsT9(np.asarray(inputs["s2_wg"], np.float64)),
        lhsT9(wf2)], axis=2).astype(np.float16)            # [128, 9, 49]
    W0T = np.ascontiguousarray(np.asarray(inputs["conv0_w"], np.float32).T
                               .reshape(12, 128, 8).transpose(1, 0, 2)).astype(np.float16)
    WSM = np.concatenate([
        np.asarray(inputs["conv1_w"], np.float32).T.reshape(-1),
        np.asarray(inputs["conv2_w"], np.float32).T.reshape(-1)]).astype(np.float16)  # [144]
    BS = np.ascontiguousarray(np.stack([inputs["s0_bs"], inputs["s1_bs"],
                                        inputs["s2_bs"]]).T).astype(np.float32)  # [128,3]
    GB0 = np.ascontiguousarray((1.0 + np.asarray(inputs["s0_bg"], np.float32))
                               .reshape(12, 128).T).astype(np.float32)           # [128,12]
    BIASV = np.concatenate([
        1.0 + np.asarray(inputs["s1_bg"], np.float64),
        1.0 + np.asarray(inputs["s2_bg"], np.float64),
        np.asarray(inputs["b0"], np.float64)
        + np.asarray(inputs["conv0_w"], np.float64) @ np.asarray(inputs["s0_bb"], np.float64),
        np.asarray(inputs["b1"], np.float64)
        + np.asarray(inputs["conv1_w"], np.float64) @ np.asarray(inputs["s1_bb"], np.float64),
        np.asarray(inputs["b2"], np.float64)
        + np.asarray(inputs["conv2_w"], np.float64) @ np.asarray(inputs["s2_bb"], np.float64),
    ]).astype(np.float32)                                   # [49]

    shared = dict(ws=WS, wg=WG, wsm9=WSM9, w0t=W0T, wsm=WSM, bs=BS, gb0=GB0,
                  biasv=BIASV)

    in_maps = []
    for core in range(8):
        k, h = core // 2, core % 2
        r0 = HROWS * h
        X = np.ascontiguousarray(
            x_main[k, :, r0:r0 + HROWS, :].reshape(12, 128, 512).transpose(1, 0, 2)
        ).astype(np.float16)
        FT = np.ascontiguousarray(
            f_sem[k].reshape(384, NPOS).T.reshape(8, 128, 384).transpose(1, 0, 2)
        ).astype(np.float16)
        ids_flat = seg[k, ::14, ::14].astype(np.float32).reshape(NPOS)
        IDS = np.ascontiguousarray(ids_flat.reshape(8, 128).T)
        rows = np.arange(r0 - 2, r0 + HROWS + 2)          # 20 sm rows
        valid = (rows >= 0) & (rows < Hp)
        rcl = np.clip(rows, 0, Hp - 1)
        cid = np.empty((SMR, Wp, 4), np.float32)
        cols = np.arange(Wp)
        for t, (dy, dx) in enumerate([(0, 0), (0, 1), (1, 0), (1, 1)]):
            v = seg[k][np.ix_(14 * rcl + 6 + dy, 14 * cols + 6 + dx)].astype(np.float32)
            v[~valid, :] = -1.0
            cid[:, :, t] = v
        CID = np.ascontiguousarray(cid.reshape(5, 128, 4).transpose(1, 0, 2))
        hrows = np.arange(r0 - 1, r0 + HROWS + 1)
        HM = ((hrows >= 0) & (hrows < Hp)).astype(np.float16)
        in_maps.append(dict(shared, x=X, ft=FT, ids=IDS, cid=CID, hmask=HM))
    return in_maps


def kernel(**inputs):
    global _BUILT, LAST_RESULTS
    if _BUILT is None:
        _BUILT = _build_nc()
    nc = _BUILT
    in_maps = _host_prep(inputs)
    trace = bool(os.environ.get("BASS_TRACE"))
    res = run_bass_kernel_spmd(nc, in_maps, list(range(8)), trace=trace)
    LAST_RESULTS = res
    out = np.empty((B, 1, Hp, Wp), np.float32)
    for core in range(8):
        k, h = core // 2, core % 2
        out[k, 0, HROWS * h:HROWS * (h + 1), :] = \
            res.results[core]["out_half"].reshape(HROWS, Wp)
    return out

